# revision 16
# baseline (speedup 1.0000x reference)
# GRU summary kernel for Trainium2 (Bass/Tile), 8-core data-parallel over batch.
#
# Reference computation (see problem spec):
#   xp = x * W + b_i                      (rank-1 input projection, x scalar/step)
#   per t: rec = h @ U + b_r
#          z = sig(xp_z + rec_z); r = sig(xp_r + rec_r)
#          hh = tanh(xp_h + r * rec_h);  h = z*h + (1-z)*hh
#   out = LN(h) @ Wd + bd
#
# Layout: everything transposed ("f2"): state hT[p, c*64+b] = h[b, c*128+p],
# so matmul outputs (recT) land in [128-partition, batch-free] tiles and no
# per-step transposes are needed. U blocks are the stationary operand (bf16,
# FWL), hT is the moving operand. The rank-1 x-projection rides as K=2 seed
# matmuls with stationary [W_chunk; bias_chunk] and moving [x_t; 1].
#
# The per-step serial chain (the kernel is latency-bound, ~2.1-2.3us/step):
#   tanh -> u2=w*hh (DVE mult, w=sig(-pz) via ACT scale=-1) -> 4 u2-mains
#   (PE) -> sig_r (ACT) -> rr=r*rec_h (DVE) -> ha=rr+xh (DVE) -> tanh ...
# Everything else (sig_z, w=1-z, v=z*h, h=v+u2, z/b-mains, seeds, DMA) is
# scheduled into the chain's engine-idle windows. Engine queue order is
# pinned with explicit dependency edges where the Tile scheduler's cost
# model would otherwise interleave off-chain ops into the chain (see the
# add_dependency block below).
import os
from contextlib import ExitStack

import numpy as np
import ml_dtypes

import concourse.bass as bass
import concourse.tile as tile
from concourse import bacc, mybir
from concourse.bass import ts
from concourse.bass_utils import run_bass_kernel_spmd

B, T, UH, S = 512, 1024, 256, 16
NCORES = 8
BC = B // NCORES  # 64 batch rows per core
QW = 256          # steps per window (fully unrolled inside For_i body)
NCH = 8           # xh DMA chunks per window (pipelines the 8.4MB load)
CH = QW // NCH
LN_EPS = 1e-3

F32 = mybir.dt.float32
BF16 = mybir.dt.bfloat16
AF = mybir.ActivationFunctionType
OP = mybir.AluOpType

# number of windows; For_i loops over these. Overridable for smoke tests.
NW = T // QW


def _build(nc: bacc.Bacc, nw: int, br3_zero: bool):
    t_total = nw * QW
    x1_d = nc.dram_tensor("x1", [2, T, BC], BF16, kind="ExternalInput")
    xh_d = nc.dram_tensor("xh3", [128, T, 2, BC], BF16, kind="ExternalInput")
    ub_d = nc.dram_tensor("ub", [128, 12, 128], BF16, kind="ExternalInput")
    ubn_d = nc.dram_tensor("ubn", [128, 4, 128], BF16, kind="ExternalInput")
    wb_d = nc.dram_tensor("wb", [2, 6, 128], BF16, kind="ExternalInput")
    br3_d = nc.dram_tensor("br3", [128, 2], F32, kind="ExternalInput")
    gb_d = nc.dram_tensor("gb", [128, 4], F32, kind="ExternalInput")
    wd_d = nc.dram_tensor("wd", [128, 2, S], F32, kind="ExternalInput")
    bd_d = nc.dram_tensor("bd", [1, S], F32, kind="ExternalInput")
    out_d = nc.dram_tensor("out", [BC, S], F32, kind="ExternalOutput")

    with ExitStack() as ctx:
        tc = ctx.enter_context(tile.TileContext(nc))
        singles = ctx.enter_context(tc.tile_pool(name="singles", bufs=1))
        # bufs=1: For_i ends each window with an all-engine barrier, so
        # cross-window DMA/compute overlap is impossible anyway; one buffer
        # halves the SBUF footprint (QW=256 -> 64KB/partition for xh).
        xwin = ctx.enter_context(tc.tile_pool(name="xwin", bufs=1))
        psum = ctx.enter_context(tc.tile_pool(name="psum", bufs=2, space="PSUM"))
        psum1 = ctx.enter_context(tc.tile_pool(name="psum1", bufs=1, space="PSUM"))
        work = ctx.enter_context(tc.tile_pool(name="work", bufs=3))

        ub_s = singles.tile([128, 12, 128], BF16)
        nc.sync.dma_start(out=ub_s, in_=ub_d.ap())
        ubn_s = singles.tile([128, 4, 128], BF16)
        nc.sync.dma_start(out=ubn_s, in_=ubn_d.ap())
        wb_s = singles.tile([2, 6, 128], BF16)
        nc.sync.dma_start(out=wb_s, in_=wb_d.ap())
        br3_s = singles.tile([128, 2], F32)
        nc.sync.dma_start(out=br3_s, in_=br3_d.ap())
        gb_s = singles.tile([128, 4], F32)
        nc.sync.dma_start(out=gb_s, in_=gb_d.ap())
        wd_s = singles.tile([128, 2, S], F32)
        nc.sync.dma_start(out=wd_s, in_=wd_d.ap())
        bd_s = singles.tile([1, S], F32)
        nc.sync.dma_start(out=bd_s, in_=bd_d.ap())

        ones_r = singles.tile([1, 128], F32)
        nc.vector.memset(ones_r, 1.0)
        ones_c = singles.tile([128, 1], F32)
        nc.vector.memset(ones_c, 1.0)
        eps_s = singles.tile([1, 1], F32)
        nc.vector.memset(eps_s, LN_EPS)

        hb = singles.tile([128, 128], BF16)
        nc.vector.memset(hb, 0.0)
        v_prev = singles.tile([128, 128], BF16)
        nc.vector.memset(v_prev, 0.0)
        u2_prev = singles.tile([128, 128], BF16)
        nc.vector.memset(u2_prev, 0.0)

        # --- PE warm-up: ~30 back-to-back large matmuls so the HAM clock
        # gate opens (K=8/8, 2.4 GHz). The steady-state loop's PE idle gaps
        # are well under the ~3.4us MID window, so once warm it stays warm.
        warm_ps = psum1.tile([128, 512], F32, tag="warm")
        for _ in range(30):
            nc.tensor.matmul(warm_ps, ub_s[:, 0, :], ub_s[:, 0:4, :],
                             start=True, stop=True)

        def step(xs, xh, pz, pr, pb):
            # Seeds first: x-only deps, run during the previous gate chain.
            # NOTE: start=True clears has_written for the WHOLE bank -> exactly
            # one start=True per bank (its first write).
            for i, m in enumerate((2, 3)):
                nc.tensor.matmul(pr[:, i * 64:(i + 1) * 64], wb_s[0:2, m, :], xs,
                                 start=(i == 0), stop=False, skip_group_check=True)
            for m in (0, 1):
                nc.tensor.matmul(pz[:, m * 64:(m + 1) * 64], wb_s[0:2, m, :], xs,
                                 start=(m == 0), stop=False, skip_group_check=True)
            # r mains split via h_prev = v_prev + u2_prev (matmul linearity):
            # the v-part streams during the previous step's tanh; only the
            # u2-part (available right after tanh) sits on the serial chain.
            for kc in range(2):
                for i in range(2):
                    nc.tensor.matmul(pr[:, i * 64:(i + 1) * 64],
                                     ubn_s[:, 2 * kc + i, :],
                                     v_prev[:, kc * 64:(kc + 1) * 64],
                                     start=False, stop=False,
                                     skip_group_check=True)
            # u2_prev holds -u2 = (z-1)*hh (one fused stt op on the chain);
            # negated U blocks make the PSUM contribution (-U)^T(-u2) = +U^T u2.
            for kc in range(2):
                for i, m in enumerate((2, 3)):
                    nc.tensor.matmul(pr[:, i * 64:(i + 1) * 64],
                                     ub_s[:, 6 * kc + m, :],
                                     u2_prev[:, kc * 64:(kc + 1) * 64],
                                     start=False, stop=(kc == 1 and i == 1),
                                     skip_group_check=True)
            for kc in range(2):
                for i, m in enumerate((4, 5)):
                    nc.tensor.matmul(pb[:, i * 64:(i + 1) * 64],
                                     ub_s[:, 6 * kc + m, :],
                                     hb[:, kc * 64:(kc + 1) * 64],
                                     start=(kc == 0 and i == 0),
                                     stop=(kc == 1 and i == 1),
                                     skip_group_check=True)
            for kc in range(2):
                for m in (0, 1):
                    nc.tensor.matmul(pz[:, m * 64:(m + 1) * 64],
                                     ub_s[:, 6 * kc + m, :],
                                     hb[:, kc * 64:(kc + 1) * 64],
                                     start=False, stop=(kc == 1 and m == 1),
                                     skip_group_check=True)

            # --- gate chain: sig_r -> rrh -> ha -> tanh -> u2 -> (next r-mains)
            r_sb = work.tile([128, 128], BF16, tag="r")
            nc.scalar.activation(r_sb, pr, AF.Sigmoid)
            rr_sb = work.tile([128, 128], BF16, tag="rr")
            if br3_zero:
                nc.vector.tensor_mul(rr_sb, pb, r_sb)
            else:
                for c in range(2):
                    nc.vector.scalar_tensor_tensor(
                        rr_sb[:, c * 64:(c + 1) * 64], pb[:, c * 64:(c + 1) * 64],
                        br3_s[:, c:c + 1], r_sb[:, c * 64:(c + 1) * 64],
                        op0=OP.add, op1=OP.mult)
            ha_i = None
            ha_sb = work.tile([128, 128], BF16, tag="ha")
            ha_i = nc.vector.tensor_add(ha_sb, rr_sb, xh)
            # sig_w = 1-z = sig(-pz) via ACT's free scale=-1, emitted before
            # tanh: its release (pz stop) lands mid-chain, well before tanh's
            # input is ready.
            z_sb = work.tile([128, 128], BF16, tag="z")
            sigz_i = nc.scalar.activation(z_sb, pz, AF.Sigmoid, scale=-1.0)
            hh_sb = work.tile([128, 128], BF16, tag="hh")
            tanh_i = nc.scalar.activation(hh_sb, ha_sb, AF.Tanh)

            # off-chain: vn = (w-1)*h_prev = -z*h_prev (stt; must land early
            # enough for the next step's vn-mains, which use the negated U
            # blocks); chain: u2 = w*hh = (1-z)*hh as a plain MULT (~65ns
            # cheaper than the stt it replaces); then h = u2 - vn.
            v_i = nc.vector.scalar_tensor_tensor(v_prev, z_sb, 1.0, hb,
                                                 op0=OP.subtract, op1=OP.mult)
            u2_i = nc.vector.tensor_mul(u2_prev, z_sb, hh_sb)
            nc.vector.tensor_sub(hb, u2_prev, v_prev)
            # Pin the DVE queue order ha -> v -> u2n and the ACT order
            # sig_z -> tanh. Without these edges the Tile scheduler's cost
            # model sometimes slots v between rr and ha (+~300ns of chain per
            # step, measured on v1), or parks sig_z after tanh, stalling v.
            _dep = v_i.ins.get_dependency_info(v_i.ins.sync_dependency_names()[0])
            v_i.ins.add_dependency(ha_i.ins.name, _dep)
            u2_i.ins.add_dependency(v_i.ins.name, _dep)
            tanh_i.ins.add_dependency(sigz_i.ins.name, _dep)

            # Dummy matmuls (stationary = fresh chain tiles, so they schedule
            # into THIS step's chain window) keep the PE busy so the HAM clock
            # gate stays open (2.4 GHz): without them the whole chip drops to
            # a lower clock state (every ACT/DVE op measured ~1.2x slower).
            nc.tensor.matmul(warm_ps, r_sb, ub_s[:, 0:4, :], start=True, stop=True)
            nc.tensor.matmul(warm_ps, r_sb, ub_s[:, 4:8, :], start=True, stop=True)
            nc.tensor.matmul(warm_ps, z_sb, ub_s[:, 0:4, :], start=True, stop=True)
            nc.tensor.matmul(warm_ps, z_sb, ub_s[:, 4:8, :], start=True, stop=True)

        with tc.For_i(0, nw, hint_engines=(mybir.EngineType.PE,
                                            mybir.EngineType.Activation,
                                            mybir.EngineType.DVE)) as w:
            xw = xwin.tile([2, QW, BC], BF16, tag="xw")
            nc.sync.dma_start(out=xw, in_=x1_d.ap()[:, ts(w, QW), :])
            # xh window load split into NCH chunks so step q only waits for
            # chunk q//CH (the 8.4MB whole-window load cost ~30us serial).
            xh_tiles = []
            for k in range(NCH):
                t3 = xwin.tile([128, CH, 2, BC], BF16, tag=f"xh{k}")
                nc.sync.dma_start(out=t3, in_=xh_d.ap()[:, ts(w * NCH + k, CH), :, :])
                xh_tiles.append(t3)
            for q in range(QW):
                pz = psum.tile([128, 128], F32, tag="pz")
                pr = psum.tile([128, 128], F32, tag="pr")
                pb = psum.tile([128, 128], F32, tag="pb")
                step(xw[0:2, q, :], xh_tiles[q // CH][:, q % CH, :, :], pz, pr, pb)

        # ---- epilogue: LayerNorm over hidden dim (partition axis) + dense
        ones_cb = singles.tile([128, 1], BF16)
        nc.vector.memset(ones_cb, 1.0)
        sq = work.tile([128, 128], F32, tag="sq")
        nc.vector.tensor_mul(sq, hb, hb)
        ps1 = psum.tile([1, 128], F32, tag="pz")
        nc.tensor.matmul(ps1, ones_cb, hb, start=True, stop=True)
        ps2 = psum.tile([1, 128], F32, tag="pb")
        nc.tensor.matmul(ps2, ones_c, sq, start=True, stop=True)

        s1_sb = work.tile([1, 128], F32, tag="s1")
        nc.vector.tensor_copy(s1_sb, ps1)
        s2_sb = work.tile([1, 128], F32, tag="s2")
        nc.vector.tensor_copy(s2_sb, ps2)
        mean_r = work.tile([1, 64], F32, tag="mean")
        nc.vector.tensor_add(mean_r, s1_sb[0:1, 0:64], s1_sb[0:1, 64:128])
        nc.vector.tensor_scalar_mul(mean_r, mean_r, 1.0 / UH)
        msq_r = work.tile([1, 64], F32, tag="msq")
        nc.vector.tensor_add(msq_r, s2_sb[0:1, 0:64], s2_sb[0:1, 64:128])
        nc.vector.tensor_scalar_mul(msq_r, msq_r, 1.0 / UH)
        m2_r = work.tile([1, 64], F32, tag="m2")
        nc.vector.tensor_mul(m2_r, mean_r, mean_r)
        var_r = work.tile([1, 64], F32, tag="var")
        nc.vector.tensor_sub(var_r, msq_r, m2_r)
        std_r = work.tile([1, 64], F32, tag="std")
        nc.scalar.activation(std_r, var_r, AF.Sqrt, bias=eps_s)
        rstd_r = work.tile([1, 64], F32, tag="rstd")
        nc.vector.reciprocal(rstd_r, std_r)

        pk = work.tile([1, 128], F32, tag="pk")
        nc.vector.tensor_copy(pk[0:1, 0:64], mean_r)
        nc.vector.tensor_copy(pk[0:1, 64:128], rstd_r)
        pbc = psum.tile([128, 128], F32, tag="pr")
        nc.tensor.matmul(pbc, ones_r, pk, start=True, stop=True)

        hn = work.tile([128, 128], F32, tag="hn")
        for c in range(2):
            t1 = work.tile([128, 64], F32, tag="t1")
            nc.vector.tensor_sub(t1, hb[:, c * 64:(c + 1) * 64], pbc[:, 0:64])
            t2 = work.tile([128, 64], F32, tag="t2")
            nc.vector.tensor_mul(t2, t1, pbc[:, 64:128])
            nc.vector.tensor_scalar(hn[:, c * 64:(c + 1) * 64], t2,
                                    gb_s[:, c:c + 1], gb_s[:, 2 + c:3 + c],
                                    op0=OP.mult, op1=OP.add)

        pd = psum.tile([64, S], F32, tag="pz")
        nc.tensor.matmul(pd, hn[:, 0:64], wd_s[:, 0, :], start=True, stop=False)
        nc.tensor.matmul(pd, hn[:, 64:128], wd_s[:, 1, :], start=False, stop=False)
        nc.tensor.matmul(pd, ones_r[0:1, 0:64], bd_s, start=False, stop=True)
        ob = work.tile([64, S], F32, tag="ob")
        nc.vector.tensor_copy(ob, pd)
        nc.sync.dma_start(out=out_d.ap(), in_=ob)


def kernel(**inputs) -> np.ndarray:
    x = np.asarray(inputs["time_series"], np.float32)[:, :, 0]  # (512, 1024)
    W = np.asarray(inputs["W"], np.float32)[0]                  # (768,)
    U = np.asarray(inputs["U"], np.float32)                     # (256, 768)
    b_i = np.asarray(inputs["b_i"], np.float32)
    b_r = np.asarray(inputs["b_r"], np.float32)
    ln_gamma = np.asarray(inputs["ln_gamma"], np.float32)
    ln_beta = np.asarray(inputs["ln_beta"], np.float32)
    Wd = np.asarray(inputs["Wd"], np.float32)
    bd = np.asarray(inputs["bd"], np.float32)

    nw = int(os.environ.get("GRU_NW", NW))
    br3_zero = not np.any(b_r[512:768])

    nc = bacc.Bacc("TRN2", target_bir_lowering=False, debug=False,
                   enable_asserts=True, num_devices=NCORES)
    _build(nc, nw, br3_zero)
    nc.compile()

    bf = ml_dtypes.bfloat16
    # U blocks: ub[p, kc*6+m, j] = U[kc*128+p, m*128+j]
    ub4 = U.reshape(2, 128, 6, 128).transpose(1, 0, 2, 3)
    ub = np.ascontiguousarray(ub4.reshape(128, 12, 128)).astype(bf)
    # negated r-gate blocks (m=2,3) for the u2n matmuls: (-U)^T(-u2) = U^T u2
    ubn = np.ascontiguousarray((-ub4[:, :, 2:4, :]).reshape(128, 4, 128)).astype(bf)
    # seed stationaries: [W_chunk; bias_chunk]
    wb = np.empty((2, 6, 128), np.float32)
    wb[0] = W.reshape(6, 128)
    bsum = b_i + b_r
    wb[1, 0:4] = bsum[:512].reshape(4, 128)
    wb[1, 4:6] = b_i[512:].reshape(2, 128)
    wb = wb.astype(bf)
    br3 = np.ascontiguousarray(b_r[512:].reshape(2, 128).T)  # [p, c]
    gb = np.empty((128, 4), np.float32)
    gb[:, 0:2] = ln_gamma.reshape(2, 128).T
    gb[:, 2:4] = ln_beta.reshape(2, 128).T
    wd = np.ascontiguousarray(Wd.reshape(2, 128, S).transpose(1, 0, 2))
    bdv = np.ascontiguousarray(bd.reshape(1, S))

    W3r = W[512:].reshape(2, 128)
    bi3r = b_i[512:].reshape(2, 128)
    in_maps = []
    for c in range(NCORES):
        xc = x[c * BC:(c + 1) * BC]  # (64, 1024)
        x1 = np.empty((2, T, BC), np.float32)
        x1[0] = xc.T
        x1[1] = 1.0
        # xh3[p, t, c2, b] = W3[c2*128+p]*x[b, t] + b_i[512+c2*128+p]
        xh3 = (W3r.T[:, None, :, None] * xc.T[None, :, None, :]
               + bi3r.T[:, None, :, None]).astype(bf)
        in_maps.append({
            "x1": x1.astype(bf), "xh3": xh3, "ub": ub, "ubn": ubn, "wb": wb,
            "br3": br3, "gb": gb, "wd": wd, "bd": bdv,
        })

    trace = os.environ.get("GRU_TRACE", "") == "1"
    # The first execution of a freshly compiled NEFF occasionally hits a
    # transient NRT_EXEC_UNIT_UNRECOVERABLE on this stack; a retry succeeds.
    res = None
    last_err = None
    for attempt in range(3):
        try:
            res = run_bass_kernel_spmd(nc, in_maps, core_ids=list(range(NCORES)),
                                       trace=trace)
            break
        except Exception as e:  # noqa: BLE001
            last_err = e
    if res is None:
        raise last_err
    if trace:
        print(f"HW exec time: {res.exec_time_ns} ns")
        if res.instructions_and_trace:
            print(f"trace: {res.instructions_and_trace[1]}")
    out = np.concatenate([res.results[c]["out"] for c in range(NCORES)], axis=0)
    return out.astype(np.float32)



# revision 17
# speedup vs baseline: 1.1060x; 1.1060x over previous
# GRU summary kernel for Trainium2 (Bass/Tile), 8-core data-parallel over batch.
#
# Reference computation (see problem spec):
#   xp = x * W + b_i                      (rank-1 input projection, x scalar/step)
#   per t: rec = h @ U + b_r
#          z = sig(xp_z + rec_z); r = sig(xp_r + rec_r)
#          hh = tanh(xp_h + r * rec_h);  h = z*h + (1-z)*hh
#   out = LN(h) @ Wd + bd
#
# Layout: everything transposed ("f2"): state hT[p, c*64+b] = h[b, c*128+p],
# so matmul outputs (recT) land in [128-partition, batch-free] tiles and no
# per-step transposes are needed. U blocks are the stationary operand (bf16,
# FWL), hT is the moving operand. The rank-1 x-projection rides as K=2 seed
# matmuls with stationary [W_chunk; bias_chunk] and moving [x_t; 1].
#
# The per-step serial chain (the kernel is latency-bound, ~2.1-2.3us/step):
#   tanh -> u2=w*hh (DVE) -> 4 u2-mains (PE) -> sig_r (ACT) -> rr=r*rec_h
#   (DVE) -> ha=rr+xh (DVE) -> tanh ...
# Everything else (sig_z, w=1-z, v=z*h, h=v+u2, z/b-mains, seeds, DMA) is
# scheduled into the chain's engine-idle windows. Engine queue order is
# pinned with explicit dependency edges where the Tile scheduler's cost
# model would otherwise interleave off-chain ops into the chain (see the
# add_dependency block below).
import os
from contextlib import ExitStack

import numpy as np
import ml_dtypes

import concourse.bass as bass
import concourse.tile as tile
from concourse import bacc, mybir
from concourse.bass import ts
from concourse.bass_utils import run_bass_kernel_spmd

B, T, UH, S = 512, 1024, 256, 16
NCORES = 8
BC = B // NCORES  # 64 batch rows per core
QW = 256          # steps per window (fully unrolled inside For_i body)
NCH = 8           # xh DMA chunks per window (pipelines the 8.4MB load)
CH = QW // NCH
LN_EPS = 1e-3

F32 = mybir.dt.float32
BF16 = mybir.dt.bfloat16
AF = mybir.ActivationFunctionType
OP = mybir.AluOpType

# number of windows; For_i loops over these. Overridable for smoke tests.
NW = T // QW


def _build(nc: bacc.Bacc, nw: int, br3_zero: bool):
    t_total = nw * QW
    x1_d = nc.dram_tensor("x1", [2, T, BC], BF16, kind="ExternalInput")
    xh_d = nc.dram_tensor("xh3", [128, T, 2, BC], BF16, kind="ExternalInput")
    ub_d = nc.dram_tensor("ub", [128, 12, 128], BF16, kind="ExternalInput")
    ubn_d = nc.dram_tensor("ubn", [128, 4, 128], BF16, kind="ExternalInput")
    wb_d = nc.dram_tensor("wb", [2, 6, 128], BF16, kind="ExternalInput")
    br3_d = nc.dram_tensor("br3", [128, 2], F32, kind="ExternalInput")
    gb_d = nc.dram_tensor("gb", [128, 4], F32, kind="ExternalInput")
    wd_d = nc.dram_tensor("wd", [128, 2, S], F32, kind="ExternalInput")
    bd_d = nc.dram_tensor("bd", [1, S], F32, kind="ExternalInput")
    out_d = nc.dram_tensor("out", [BC, S], F32, kind="ExternalOutput")

    with ExitStack() as ctx:
        tc = ctx.enter_context(tile.TileContext(nc))
        singles = ctx.enter_context(tc.tile_pool(name="singles", bufs=1))
        # bufs=1: For_i ends each window with an all-engine barrier, so
        # cross-window DMA/compute overlap is impossible anyway; one buffer
        # halves the SBUF footprint (QW=256 -> 64KB/partition for xh).
        xwin = ctx.enter_context(tc.tile_pool(name="xwin", bufs=1))
        psum = ctx.enter_context(tc.tile_pool(name="psum", bufs=2, space="PSUM"))
        psum1 = ctx.enter_context(tc.tile_pool(name="psum1", bufs=1, space="PSUM"))
        work = ctx.enter_context(tc.tile_pool(name="work", bufs=3))

        ub_s = singles.tile([128, 12, 128], BF16)
        nc.sync.dma_start(out=ub_s, in_=ub_d.ap())
        ubn_s = singles.tile([128, 4, 128], BF16)
        nc.sync.dma_start(out=ubn_s, in_=ubn_d.ap())
        wb_s = singles.tile([2, 6, 128], BF16)
        nc.sync.dma_start(out=wb_s, in_=wb_d.ap())
        br3_s = singles.tile([128, 2], F32)
        nc.sync.dma_start(out=br3_s, in_=br3_d.ap())
        gb_s = singles.tile([128, 4], F32)
        nc.sync.dma_start(out=gb_s, in_=gb_d.ap())
        wd_s = singles.tile([128, 2, S], F32)
        nc.sync.dma_start(out=wd_s, in_=wd_d.ap())
        bd_s = singles.tile([1, S], F32)
        nc.sync.dma_start(out=bd_s, in_=bd_d.ap())

        ones_r = singles.tile([1, 128], F32)
        nc.vector.memset(ones_r, 1.0)
        ones_c = singles.tile([128, 1], F32)
        nc.vector.memset(ones_c, 1.0)
        eps_s = singles.tile([1, 1], F32)
        nc.vector.memset(eps_s, LN_EPS)

        hb = singles.tile([128, 128], BF16)
        nc.vector.memset(hb, 0.0)
        v_prev = singles.tile([128, 128], BF16)
        nc.vector.memset(v_prev, 0.0)
        u2_prev = singles.tile([128, 128], BF16)
        nc.vector.memset(u2_prev, 0.0)

        # --- PE warm-up: ~30 back-to-back large matmuls so the HAM clock
        # gate opens (K=8/8, 2.4 GHz). The steady-state loop's PE idle gaps
        # are well under the ~3.4us MID window, so once warm it stays warm.
        warm_ps = psum1.tile([128, 512], F32, tag="warm")
        for _ in range(30):
            nc.tensor.matmul(warm_ps, ub_s[:, 0, :], ub_s[:, 0:4, :],
                             start=True, stop=True)

        def step(xs, xh, pz, pr, pb):
            # Seeds first: x-only deps, run during the previous gate chain.
            # NOTE: start=True clears has_written for the WHOLE bank -> exactly
            # one start=True per bank (its first write).
            for i, m in enumerate((2, 3)):
                nc.tensor.matmul(pr[:, i * 64:(i + 1) * 64], wb_s[0:2, m, :], xs,
                                 start=(i == 0), stop=False, skip_group_check=True)
            for m in (0, 1):
                nc.tensor.matmul(pz[:, m * 64:(m + 1) * 64], wb_s[0:2, m, :], xs,
                                 start=(m == 0), stop=False, skip_group_check=True)
            # r mains split via h_prev = v_prev + u2_prev (matmul linearity):
            # the v-part streams during the previous step's tanh; only the
            # u2-part (available right after tanh) sits on the serial chain.
            for kc in range(2):
                for i, m in enumerate((2, 3)):
                    nc.tensor.matmul(pr[:, i * 64:(i + 1) * 64],
                                     ub_s[:, 6 * kc + m, :],
                                     v_prev[:, kc * 64:(kc + 1) * 64],
                                     start=False, stop=False,
                                     skip_group_check=True)
            # u2_prev holds -u2 = (z-1)*hh (one fused stt op on the chain);
            # negated U blocks make the PSUM contribution (-U)^T(-u2) = +U^T u2.
            for kc in range(2):
                for i in range(2):
                    nc.tensor.matmul(pr[:, i * 64:(i + 1) * 64],
                                     ubn_s[:, 2 * kc + i, :],
                                     u2_prev[:, kc * 64:(kc + 1) * 64],
                                     start=False, stop=(kc == 1 and i == 1),
                                     skip_group_check=True)
            for kc in range(2):
                for i, m in enumerate((4, 5)):
                    nc.tensor.matmul(pb[:, i * 64:(i + 1) * 64],
                                     ub_s[:, 6 * kc + m, :],
                                     hb[:, kc * 64:(kc + 1) * 64],
                                     start=(kc == 0 and i == 0),
                                     stop=(kc == 1 and i == 1),
                                     skip_group_check=True)
            for kc in range(2):
                for m in (0, 1):
                    nc.tensor.matmul(pz[:, m * 64:(m + 1) * 64],
                                     ub_s[:, 6 * kc + m, :],
                                     hb[:, kc * 64:(kc + 1) * 64],
                                     start=False, stop=(kc == 1 and m == 1),
                                     skip_group_check=True)

            # --- gate chain: sig_r -> rrh -> ha -> tanh -> u2 -> (next r-mains)
            r_sb = work.tile([128, 128], BF16, tag="r")
            nc.scalar.activation(r_sb, pr, AF.Sigmoid)
            rr_sb = work.tile([128, 128], BF16, tag="rr")
            if br3_zero:
                nc.vector.tensor_mul(rr_sb, pb, r_sb)
            else:
                for c in range(2):
                    nc.vector.scalar_tensor_tensor(
                        rr_sb[:, c * 64:(c + 1) * 64], pb[:, c * 64:(c + 1) * 64],
                        br3_s[:, c:c + 1], r_sb[:, c * 64:(c + 1) * 64],
                        op0=OP.add, op1=OP.mult)
            ha_i = None
            ha_sb = work.tile([128, 128], BF16, tag="ha")
            ha_i = nc.vector.tensor_add(ha_sb, rr_sb, xh)
            # sig_z emitted before tanh: its release (pz stop) lands
            # mid-chain, well before tanh's input is ready.
            z_sb = work.tile([128, 128], BF16, tag="z")
            sigz_i = nc.scalar.activation(z_sb, pz, AF.Sigmoid)
            hh_sb = work.tile([128, 128], BF16, tag="hh")
            tanh_i = nc.scalar.activation(hh_sb, ha_sb, AF.Tanh)

            # off-chain: v = z*h_prev (must land early enough for the next
            # step's v-mains); chain: u2n = (z-1)*hh = -u2 in ONE fused stt
            # (no separate w=1-z op, so only v occupies the DVE during tanh
            # and u2n issues right at tanh_end+sem); then h = v - u2n.
            v_i = nc.vector.tensor_mul(v_prev, z_sb, hb)
            u2_i = nc.vector.scalar_tensor_tensor(u2_prev, z_sb, 1.0, hh_sb,
                                                  op0=OP.subtract, op1=OP.mult)
            nc.vector.tensor_sub(hb, v_prev, u2_prev)
            # Pin the DVE queue order ha -> v -> u2n and the ACT order
            # sig_z -> tanh. Without these edges the Tile scheduler's cost
            # model sometimes slots v between rr and ha (+~300ns of chain per
            # step, measured on v1), or parks sig_z after tanh, stalling v.
            _dep = v_i.ins.get_dependency_info(v_i.ins.sync_dependency_names()[0])
            v_i.ins.add_dependency(ha_i.ins.name, _dep)
            u2_i.ins.add_dependency(v_i.ins.name, _dep)
            tanh_i.ins.add_dependency(sigz_i.ins.name, _dep)

            # Dummy matmuls (stationary = fresh chain tiles, so they schedule
            # into THIS step's chain window) keep the PE busy so the HAM clock
            # gate stays open (2.4 GHz): without them the whole chip drops to
            # a lower clock state (every ACT/DVE op measured ~1.2x slower).
            nc.tensor.matmul(warm_ps, r_sb, ub_s[:, 0:4, :], start=True, stop=True)
            nc.tensor.matmul(warm_ps, r_sb, ub_s[:, 4:8, :], start=True, stop=True)
            nc.tensor.matmul(warm_ps, z_sb, ub_s[:, 0:4, :], start=True, stop=True)
            nc.tensor.matmul(warm_ps, z_sb, ub_s[:, 4:8, :], start=True, stop=True)

        with tc.For_i(0, nw, hint_engines=(mybir.EngineType.PE,
                                            mybir.EngineType.Activation,
                                            mybir.EngineType.DVE)) as w:
            xw = xwin.tile([2, QW, BC], BF16, tag="xw")
            nc.sync.dma_start(out=xw, in_=x1_d.ap()[:, ts(w, QW), :])
            # xh window load split into NCH chunks so step q only waits for
            # chunk q//CH (the 8.4MB whole-window load cost ~30us serial).
            xh_tiles = []
            for k in range(NCH):
                t3 = xwin.tile([128, CH, 2, BC], BF16, tag=f"xh{k}")
                nc.sync.dma_start(out=t3, in_=xh_d.ap()[:, ts(w * NCH + k, CH), :, :])
                xh_tiles.append(t3)
            for q in range(QW):
                pz = psum.tile([128, 128], F32, tag="pz")
                pr = psum.tile([128, 128], F32, tag="pr")
                pb = psum.tile([128, 128], F32, tag="pb")
                step(xw[0:2, q, :], xh_tiles[q // CH][:, q % CH, :, :], pz, pr, pb)

        # ---- epilogue: LayerNorm over hidden dim (partition axis) + dense
        ones_cb = singles.tile([128, 1], BF16)
        nc.vector.memset(ones_cb, 1.0)
        sq = work.tile([128, 128], F32, tag="sq")
        nc.vector.tensor_mul(sq, hb, hb)
        ps1 = psum.tile([1, 128], F32, tag="pz")
        nc.tensor.matmul(ps1, ones_cb, hb, start=True, stop=True)
        ps2 = psum.tile([1, 128], F32, tag="pb")
        nc.tensor.matmul(ps2, ones_c, sq, start=True, stop=True)

        s1_sb = work.tile([1, 128], F32, tag="s1")
        nc.vector.tensor_copy(s1_sb, ps1)
        s2_sb = work.tile([1, 128], F32, tag="s2")
        nc.vector.tensor_copy(s2_sb, ps2)
        mean_r = work.tile([1, 64], F32, tag="mean")
        nc.vector.tensor_add(mean_r, s1_sb[0:1, 0:64], s1_sb[0:1, 64:128])
        nc.vector.tensor_scalar_mul(mean_r, mean_r, 1.0 / UH)
        msq_r = work.tile([1, 64], F32, tag="msq")
        nc.vector.tensor_add(msq_r, s2_sb[0:1, 0:64], s2_sb[0:1, 64:128])
        nc.vector.tensor_scalar_mul(msq_r, msq_r, 1.0 / UH)
        m2_r = work.tile([1, 64], F32, tag="m2")
        nc.vector.tensor_mul(m2_r, mean_r, mean_r)
        var_r = work.tile([1, 64], F32, tag="var")
        nc.vector.tensor_sub(var_r, msq_r, m2_r)
        std_r = work.tile([1, 64], F32, tag="std")
        nc.scalar.activation(std_r, var_r, AF.Sqrt, bias=eps_s)
        rstd_r = work.tile([1, 64], F32, tag="rstd")
        nc.vector.reciprocal(rstd_r, std_r)

        pk = work.tile([1, 128], F32, tag="pk")
        nc.vector.tensor_copy(pk[0:1, 0:64], mean_r)
        nc.vector.tensor_copy(pk[0:1, 64:128], rstd_r)
        pbc = psum.tile([128, 128], F32, tag="pr")
        nc.tensor.matmul(pbc, ones_r, pk, start=True, stop=True)

        hn = work.tile([128, 128], F32, tag="hn")
        for c in range(2):
            t1 = work.tile([128, 64], F32, tag="t1")
            nc.vector.tensor_sub(t1, hb[:, c * 64:(c + 1) * 64], pbc[:, 0:64])
            t2 = work.tile([128, 64], F32, tag="t2")
            nc.vector.tensor_mul(t2, t1, pbc[:, 64:128])
            nc.vector.tensor_scalar(hn[:, c * 64:(c + 1) * 64], t2,
                                    gb_s[:, c:c + 1], gb_s[:, 2 + c:3 + c],
                                    op0=OP.mult, op1=OP.add)

        pd = psum.tile([64, S], F32, tag="pz")
        nc.tensor.matmul(pd, hn[:, 0:64], wd_s[:, 0, :], start=True, stop=False)
        nc.tensor.matmul(pd, hn[:, 64:128], wd_s[:, 1, :], start=False, stop=False)
        nc.tensor.matmul(pd, ones_r[0:1, 0:64], bd_s, start=False, stop=True)
        ob = work.tile([64, S], F32, tag="ob")
        nc.vector.tensor_copy(ob, pd)
        nc.sync.dma_start(out=out_d.ap(), in_=ob)


def kernel(**inputs) -> np.ndarray:
    x = np.asarray(inputs["time_series"], np.float32)[:, :, 0]  # (512, 1024)
    W = np.asarray(inputs["W"], np.float32)[0]                  # (768,)
    U = np.asarray(inputs["U"], np.float32)                     # (256, 768)
    b_i = np.asarray(inputs["b_i"], np.float32)
    b_r = np.asarray(inputs["b_r"], np.float32)
    ln_gamma = np.asarray(inputs["ln_gamma"], np.float32)
    ln_beta = np.asarray(inputs["ln_beta"], np.float32)
    Wd = np.asarray(inputs["Wd"], np.float32)
    bd = np.asarray(inputs["bd"], np.float32)

    nw = int(os.environ.get("GRU_NW", NW))
    br3_zero = not np.any(b_r[512:768])

    nc = bacc.Bacc("TRN2", target_bir_lowering=False, debug=False,
                   enable_asserts=True, num_devices=NCORES)
    _build(nc, nw, br3_zero)
    nc.compile()

    bf = ml_dtypes.bfloat16
    # U blocks: ub[p, kc*6+m, j] = U[kc*128+p, m*128+j]
    ub4 = U.reshape(2, 128, 6, 128).transpose(1, 0, 2, 3)
    ub = np.ascontiguousarray(ub4.reshape(128, 12, 128)).astype(bf)
    # negated r-gate blocks (m=2,3) for the u2n matmuls: (-U)^T(-u2) = U^T u2
    ubn = np.ascontiguousarray((-ub4[:, :, 2:4, :]).reshape(128, 4, 128)).astype(bf)
    # seed stationaries: [W_chunk; bias_chunk]
    wb = np.empty((2, 6, 128), np.float32)
    wb[0] = W.reshape(6, 128)
    bsum = b_i + b_r
    wb[1, 0:4] = bsum[:512].reshape(4, 128)
    wb[1, 4:6] = b_i[512:].reshape(2, 128)
    wb = wb.astype(bf)
    br3 = np.ascontiguousarray(b_r[512:].reshape(2, 128).T)  # [p, c]
    gb = np.empty((128, 4), np.float32)
    gb[:, 0:2] = ln_gamma.reshape(2, 128).T
    gb[:, 2:4] = ln_beta.reshape(2, 128).T
    wd = np.ascontiguousarray(Wd.reshape(2, 128, S).transpose(1, 0, 2))
    bdv = np.ascontiguousarray(bd.reshape(1, S))

    W3r = W[512:].reshape(2, 128)
    bi3r = b_i[512:].reshape(2, 128)
    in_maps = []
    for c in range(NCORES):
        xc = x[c * BC:(c + 1) * BC]  # (64, 1024)
        x1 = np.empty((2, T, BC), np.float32)
        x1[0] = xc.T
        x1[1] = 1.0
        # xh3[p, t, c2, b] = W3[c2*128+p]*x[b, t] + b_i[512+c2*128+p]
        xh3 = (W3r.T[:, None, :, None] * xc.T[None, :, None, :]
               + bi3r.T[:, None, :, None]).astype(bf)
        in_maps.append({
            "x1": x1.astype(bf), "xh3": xh3, "ub": ub, "ubn": ubn, "wb": wb,
            "br3": br3, "gb": gb, "wd": wd, "bd": bdv,
        })

    trace = os.environ.get("GRU_TRACE", "") == "1"
    # The first execution of a freshly compiled NEFF occasionally hits a
    # transient NRT_EXEC_UNIT_UNRECOVERABLE on this stack; a retry succeeds.
    res = None
    last_err = None
    for attempt in range(3):
        try:
            res = run_bass_kernel_spmd(nc, in_maps, core_ids=list(range(NCORES)),
                                       trace=trace)
            break
        except Exception as e:  # noqa: BLE001
            last_err = e
    if res is None:
        raise last_err
    if trace:
        print(f"HW exec time: {res.exec_time_ns} ns")
        if res.instructions_and_trace:
            print(f"trace: {res.instructions_and_trace[1]}")
    out = np.concatenate([res.results[c]["out"] for c in range(NCORES)], axis=0)
    return out.astype(np.float32)



# revision 31
# speedup vs baseline: 1.1130x; 1.0063x over previous
# GRU summary kernel for Trainium2 (Bass/Tile), 8-core data-parallel over batch.
#
# Reference computation (see problem spec):
#   xp = x * W + b_i                      (rank-1 input projection, x scalar/step)
#   per t: rec = h @ U + b_r
#          z = sig(xp_z + rec_z); r = sig(xp_r + rec_r)
#          hh = tanh(xp_h + r * rec_h);  h = z*h + (1-z)*hh
#   out = LN(h) @ Wd + bd
#
# Layout: everything transposed ("f2"): state hT[p, c*64+b] = h[b, c*128+p],
# so matmul outputs (recT) land in [128-partition, batch-free] tiles and no
# per-step transposes are needed. U blocks are the stationary operand (bf16,
# FWL), hT is the moving operand. The rank-1 x-projection rides as K=2 seed
# matmuls with stationary [W_chunk; bias_chunk] and moving [x_t; 1].
#
# The per-step serial chain (the kernel is latency-bound, ~2.1-2.3us/step):
#   tanh -> u2=w*hh (DVE) -> 4 u2-mains (PE) -> sig_r (ACT) -> rr=r*rec_h
#   (DVE) -> ha=rr+xh (DVE) -> tanh ...
# Everything else (sig_z, w=1-z, v=z*h, h=v+u2, z/b-mains, seeds, DMA) is
# scheduled into the chain's engine-idle windows. Engine queue order is
# pinned with explicit dependency edges where the Tile scheduler's cost
# model would otherwise interleave off-chain ops into the chain (see the
# add_dependency block below).
import os
from contextlib import ExitStack

import numpy as np
import ml_dtypes

import concourse.bass as bass
import concourse.tile as tile
from concourse import bacc, mybir
from concourse.bass import ts
from concourse.bass_utils import run_bass_kernel_spmd

B, T, UH, S = 512, 1024, 256, 16
NCORES = 8
BC = B // NCORES  # 64 batch rows per core
QW = 512          # steps per window (fully unrolled inside For_i body)
NCH = 16          # xh DMA chunks per window (pipelines the 16.8MB load)
CH = QW // NCH
LN_EPS = 1e-3

F32 = mybir.dt.float32
BF16 = mybir.dt.bfloat16
AF = mybir.ActivationFunctionType
OP = mybir.AluOpType

# number of windows; For_i loops over these. Overridable for smoke tests.
NW = T // QW


def _build(nc: bacc.Bacc, nw: int, br3_zero: bool):
    t_total = nw * QW
    # x seed operand split across the two legal matmul partition bases
    # (moving-operand base must be 0/32/64): even t at partitions 0:2, odd t
    # at 64:66, plane t//2. Halves the per-partition pool charge vs a
    # [2, T, BC] layout (which overflows SBUF at QW=512).
    x1_d = nc.dram_tensor("x1", [2, 2, T // 2, BC], BF16, kind="ExternalInput")
    xh_d = nc.dram_tensor("xh3", [128, T, 2, BC], BF16, kind="ExternalInput")
    ub_d = nc.dram_tensor("ub", [128, 12, 128], BF16, kind="ExternalInput")
    ubn_d = nc.dram_tensor("ubn", [128, 4, 128], BF16, kind="ExternalInput")
    wb_d = nc.dram_tensor("wb", [2, 6, 128], BF16, kind="ExternalInput")
    br3_d = nc.dram_tensor("br3", [128, 2], F32, kind="ExternalInput")
    gb_d = nc.dram_tensor("gb", [128, 4], F32, kind="ExternalInput")
    wd_d = nc.dram_tensor("wd", [128, 2, S], F32, kind="ExternalInput")
    bd_d = nc.dram_tensor("bd", [1, S], F32, kind="ExternalInput")
    out_d = nc.dram_tensor("out", [BC, S], F32, kind="ExternalOutput")

    with ExitStack() as ctx:
        tc = ctx.enter_context(tile.TileContext(nc))
        singles = ctx.enter_context(tc.tile_pool(name="singles", bufs=1))
        # bufs=1: For_i ends each window with an all-engine barrier, so
        # cross-window DMA/compute overlap is impossible anyway; one buffer
        # halves the SBUF footprint (QW=256 -> 64KB/partition for xh).
        xwin = ctx.enter_context(tc.tile_pool(name="xwin", bufs=1))
        psum = ctx.enter_context(tc.tile_pool(name="psum", bufs=2, space="PSUM"))
        psum1 = ctx.enter_context(tc.tile_pool(name="psum1", bufs=1, space="PSUM"))
        work = ctx.enter_context(tc.tile_pool(name="work", bufs=3))

        ub_s = singles.tile([128, 12, 128], BF16)
        nc.sync.dma_start(out=ub_s, in_=ub_d.ap())
        ubn_s = singles.tile([128, 4, 128], BF16)
        nc.sync.dma_start(out=ubn_s, in_=ubn_d.ap())
        # seed stationary replicated at partition bases 0 and 64 (matmul
        # requires stationary and moving operands to share a base partition)
        wb_s = singles.tile([128, 6, 128], BF16)
        nc.sync.dma_start(out=wb_s[0:2, :, :], in_=wb_d.ap())
        nc.sync.dma_start(out=wb_s[64:66, :, :], in_=wb_d.ap())
        br3_s = singles.tile([128, 2], F32)
        nc.sync.dma_start(out=br3_s, in_=br3_d.ap())
        gb_s = singles.tile([128, 4], F32)
        nc.sync.dma_start(out=gb_s, in_=gb_d.ap())
        wd_s = singles.tile([128, 2, S], F32)
        nc.sync.dma_start(out=wd_s, in_=wd_d.ap())
        bd_s = singles.tile([1, S], F32)
        nc.sync.dma_start(out=bd_s, in_=bd_d.ap())

        ones_r = singles.tile([1, 128], F32)
        nc.vector.memset(ones_r, 1.0)
        ones_c = singles.tile([128, 1], F32)
        nc.vector.memset(ones_c, 1.0)
        eps_s = singles.tile([1, 1], F32)
        nc.vector.memset(eps_s, LN_EPS)

        hb = singles.tile([128, 128], BF16)
        nc.vector.memset(hb, 0.0)
        v_prev = singles.tile([128, 128], BF16)
        nc.vector.memset(v_prev, 0.0)
        u2_prev = singles.tile([128, 128], BF16)
        nc.vector.memset(u2_prev, 0.0)

        # --- PE warm-up: ~30 back-to-back large matmuls so the HAM clock
        # gate opens (K=8/8, 2.4 GHz). The steady-state loop's PE idle gaps
        # are well under the ~3.4us MID window, so once warm it stays warm.
        # 14 x ~500ns (cold->mid clock) gives >3us of continuous PE work --
        # enough to open the clock gate; 30 was ~13us of serial prologue.
        warm_ps = psum1.tile([128, 512], F32, tag="warm")
        for _ in range(14):
            nc.tensor.matmul(warm_ps, ub_s[:, 0, :], ub_s[:, 0:4, :],
                             start=True, stop=True)

        def step(xs, xh, pz, pr, pb, sbase):
            # Seeds first: x-only deps, run during the previous gate chain.
            # NOTE: start=True clears has_written for the WHOLE bank -> exactly
            # one start=True per bank (its first write).
            for i, m in enumerate((2, 3)):
                nc.tensor.matmul(pr[:, i * 64:(i + 1) * 64],
                                 wb_s[sbase:sbase + 2, m, :], xs,
                                 start=(i == 0), stop=False, skip_group_check=True)
            for m in (0, 1):
                nc.tensor.matmul(pz[:, m * 64:(m + 1) * 64],
                                 wb_s[sbase:sbase + 2, m, :], xs,
                                 start=(m == 0), stop=False, skip_group_check=True)
            # r mains split via h_prev = v_prev + u2_prev (matmul linearity):
            # the v-part streams during the previous step's tanh; only the
            # u2-part (available right after tanh) sits on the serial chain.
            for kc in range(2):
                for i, m in enumerate((2, 3)):
                    nc.tensor.matmul(pr[:, i * 64:(i + 1) * 64],
                                     ub_s[:, 6 * kc + m, :],
                                     v_prev[:, kc * 64:(kc + 1) * 64],
                                     start=False, stop=False,
                                     skip_group_check=True)
            # u2_prev holds -u2 = (z-1)*hh (one fused stt op on the chain);
            # negated U blocks make the PSUM contribution (-U)^T(-u2) = +U^T u2.
            for kc in range(2):
                for i in range(2):
                    nc.tensor.matmul(pr[:, i * 64:(i + 1) * 64],
                                     ubn_s[:, 2 * kc + i, :],
                                     u2_prev[:, kc * 64:(kc + 1) * 64],
                                     start=False, stop=(kc == 1 and i == 1),
                                     skip_group_check=True)
            for kc in range(2):
                for i, m in enumerate((4, 5)):
                    nc.tensor.matmul(pb[:, i * 64:(i + 1) * 64],
                                     ub_s[:, 6 * kc + m, :],
                                     hb[:, kc * 64:(kc + 1) * 64],
                                     start=(kc == 0 and i == 0),
                                     stop=(kc == 1 and i == 1),
                                     skip_group_check=True)
            for kc in range(2):
                for m in (0, 1):
                    nc.tensor.matmul(pz[:, m * 64:(m + 1) * 64],
                                     ub_s[:, 6 * kc + m, :],
                                     hb[:, kc * 64:(kc + 1) * 64],
                                     start=False, stop=(kc == 1 and m == 1),
                                     skip_group_check=True)

            # --- gate chain: sig_r -> rrh -> ha -> tanh -> u2 -> (next r-mains)
            r_sb = work.tile([128, 128], BF16, tag="r")
            nc.scalar.activation(r_sb, pr, AF.Sigmoid)
            rr_sb = work.tile([128, 128], BF16, tag="rr")
            if br3_zero:
                nc.vector.tensor_mul(rr_sb, pb, r_sb)
            else:
                for c in range(2):
                    nc.vector.scalar_tensor_tensor(
                        rr_sb[:, c * 64:(c + 1) * 64], pb[:, c * 64:(c + 1) * 64],
                        br3_s[:, c:c + 1], r_sb[:, c * 64:(c + 1) * 64],
                        op0=OP.add, op1=OP.mult)
            ha_i = None
            ha_sb = work.tile([128, 128], BF16, tag="ha")
            ha_i = nc.vector.tensor_add(ha_sb, rr_sb, xh)
            # sig_z emitted before tanh: its release (pz stop) lands
            # mid-chain, well before tanh's input is ready.
            z_sb = work.tile([128, 128], BF16, tag="z")
            sigz_i = nc.scalar.activation(z_sb, pz, AF.Sigmoid)
            hh_sb = work.tile([128, 128], BF16, tag="hh")
            tanh_i = nc.scalar.activation(hh_sb, ha_sb, AF.Tanh)

            # off-chain: v = z*h_prev (must land early enough for the next
            # step's v-mains); chain: u2n = (z-1)*hh = -u2 in ONE fused stt
            # (no separate w=1-z op, so only v occupies the DVE during tanh
            # and u2n issues right at tanh_end+sem); then h = v - u2n.
            v_i = nc.vector.tensor_mul(v_prev, z_sb, hb)
            u2_i = nc.vector.scalar_tensor_tensor(u2_prev, z_sb, 1.0, hh_sb,
                                                  op0=OP.subtract, op1=OP.mult)
            nc.vector.tensor_sub(hb, v_prev, u2_prev)
            # Pin the DVE queue order ha -> v -> u2n and the ACT order
            # sig_z -> tanh. Without these edges the Tile scheduler's cost
            # model sometimes slots v between rr and ha (+~300ns of chain per
            # step, measured on v1), or parks sig_z after tanh, stalling v.
            _dep = v_i.ins.get_dependency_info(v_i.ins.sync_dependency_names()[0])
            v_i.ins.add_dependency(ha_i.ins.name, _dep)
            u2_i.ins.add_dependency(v_i.ins.name, _dep)
            tanh_i.ins.add_dependency(sigz_i.ins.name, _dep)

            # Dummy matmuls (stationary = fresh chain tiles, so they schedule
            # into THIS step's chain window) keep the PE busy so the HAM clock
            # gate stays open (2.4 GHz): without them the whole chip drops to
            # a lower clock state (every ACT/DVE op measured ~1.2x slower).
            nc.tensor.matmul(warm_ps, r_sb, ub_s[:, 0:4, :], start=True, stop=True)
            nc.tensor.matmul(warm_ps, r_sb, ub_s[:, 4:8, :], start=True, stop=True)
            nc.tensor.matmul(warm_ps, z_sb, ub_s[:, 0:4, :], start=True, stop=True)
            nc.tensor.matmul(warm_ps, z_sb, ub_s[:, 4:8, :], start=True, stop=True)

        with tc.For_i(0, nw, hint_engines=(mybir.EngineType.PE,
                                            mybir.EngineType.Activation,
                                            mybir.EngineType.DVE)) as w:
            xw = xwin.tile([128, QW // 2, BC], BF16, tag="xw")
            nc.sync.dma_start(out=xw[0:2, :, :],
                              in_=x1_d.ap()[0, :, ts(w, QW // 2), :])
            nc.sync.dma_start(out=xw[64:66, :, :],
                              in_=x1_d.ap()[1, :, ts(w, QW // 2), :])
            # xh window load split into NCH chunks so step q only waits for
            # chunk q//CH (the 8.4MB whole-window load cost ~30us serial).
            xh_tiles = []
            for k in range(NCH):
                t3 = xwin.tile([128, CH, 2, BC], BF16, tag=f"xh{k}")
                nc.sync.dma_start(out=t3, in_=xh_d.ap()[:, ts(w * NCH + k, CH), :, :])
                xh_tiles.append(t3)
            for q in range(QW):
                pz = psum.tile([128, 128], F32, tag="pz")
                pr = psum.tile([128, 128], F32, tag="pr")
                pb = psum.tile([128, 128], F32, tag="pb")
                base = 64 * (q % 2)
                step(xw[base:base + 2, q // 2, :],
                     xh_tiles[q // CH][:, q % CH, :, :], pz, pr, pb, base)

        # ---- epilogue: LayerNorm over hidden dim (partition axis) + dense
        ones_cb = singles.tile([128, 1], BF16)
        nc.vector.memset(ones_cb, 1.0)
        sq = work.tile([128, 128], F32, tag="sq")
        nc.vector.tensor_mul(sq, hb, hb)
        ps1 = psum.tile([1, 128], F32, tag="pz")
        nc.tensor.matmul(ps1, ones_cb, hb, start=True, stop=True)
        ps2 = psum.tile([1, 128], F32, tag="pb")
        nc.tensor.matmul(ps2, ones_c, sq, start=True, stop=True)

        s1_sb = work.tile([1, 128], F32, tag="s1")
        nc.vector.tensor_copy(s1_sb, ps1)
        s2_sb = work.tile([1, 128], F32, tag="s2")
        nc.vector.tensor_copy(s2_sb, ps2)
        mean_r = work.tile([1, 64], F32, tag="mean")
        nc.vector.tensor_add(mean_r, s1_sb[0:1, 0:64], s1_sb[0:1, 64:128])
        nc.vector.tensor_scalar_mul(mean_r, mean_r, 1.0 / UH)
        msq_r = work.tile([1, 64], F32, tag="msq")
        nc.vector.tensor_add(msq_r, s2_sb[0:1, 0:64], s2_sb[0:1, 64:128])
        nc.vector.tensor_scalar_mul(msq_r, msq_r, 1.0 / UH)
        m2_r = work.tile([1, 64], F32, tag="m2")
        nc.vector.tensor_mul(m2_r, mean_r, mean_r)
        var_r = work.tile([1, 64], F32, tag="var")
        nc.vector.tensor_sub(var_r, msq_r, m2_r)
        std_r = work.tile([1, 64], F32, tag="std")
        nc.scalar.activation(std_r, var_r, AF.Sqrt, bias=eps_s)
        rstd_r = work.tile([1, 64], F32, tag="rstd")
        nc.vector.reciprocal(rstd_r, std_r)

        pk = work.tile([1, 128], F32, tag="pk")
        nc.vector.tensor_copy(pk[0:1, 0:64], mean_r)
        nc.vector.tensor_copy(pk[0:1, 64:128], rstd_r)
        pbc = psum.tile([128, 128], F32, tag="pr")
        nc.tensor.matmul(pbc, ones_r, pk, start=True, stop=True)

        hn = work.tile([128, 128], F32, tag="hn")
        for c in range(2):
            t1 = work.tile([128, 64], F32, tag="t1")
            nc.vector.tensor_sub(t1, hb[:, c * 64:(c + 1) * 64], pbc[:, 0:64])
            t2 = work.tile([128, 64], F32, tag="t2")
            nc.vector.tensor_mul(t2, t1, pbc[:, 64:128])
            nc.vector.tensor_scalar(hn[:, c * 64:(c + 1) * 64], t2,
                                    gb_s[:, c:c + 1], gb_s[:, 2 + c:3 + c],
                                    op0=OP.mult, op1=OP.add)

        pd = psum.tile([64, S], F32, tag="pz")
        nc.tensor.matmul(pd, hn[:, 0:64], wd_s[:, 0, :], start=True, stop=False)
        nc.tensor.matmul(pd, hn[:, 64:128], wd_s[:, 1, :], start=False, stop=False)
        nc.tensor.matmul(pd, ones_r[0:1, 0:64], bd_s, start=False, stop=True)
        ob = work.tile([64, S], F32, tag="ob")
        nc.vector.tensor_copy(ob, pd)
        nc.sync.dma_start(out=out_d.ap(), in_=ob)


def kernel(**inputs) -> np.ndarray:
    x = np.asarray(inputs["time_series"], np.float32)[:, :, 0]  # (512, 1024)
    W = np.asarray(inputs["W"], np.float32)[0]                  # (768,)
    U = np.asarray(inputs["U"], np.float32)                     # (256, 768)
    b_i = np.asarray(inputs["b_i"], np.float32)
    b_r = np.asarray(inputs["b_r"], np.float32)
    ln_gamma = np.asarray(inputs["ln_gamma"], np.float32)
    ln_beta = np.asarray(inputs["ln_beta"], np.float32)
    Wd = np.asarray(inputs["Wd"], np.float32)
    bd = np.asarray(inputs["bd"], np.float32)

    nw = int(os.environ.get("GRU_NW", NW))
    br3_zero = not np.any(b_r[512:768])

    nc = bacc.Bacc("TRN2", target_bir_lowering=False, debug=False,
                   enable_asserts=True, num_devices=NCORES)
    _build(nc, nw, br3_zero)
    nc.compile()

    bf = ml_dtypes.bfloat16
    # U blocks: ub[p, kc*6+m, j] = U[kc*128+p, m*128+j]
    ub4 = U.reshape(2, 128, 6, 128).transpose(1, 0, 2, 3)
    ub = np.ascontiguousarray(ub4.reshape(128, 12, 128)).astype(bf)
    # negated r-gate blocks (m=2,3) for the u2n matmuls: (-U)^T(-u2) = U^T u2
    ubn = np.ascontiguousarray((-ub4[:, :, 2:4, :]).reshape(128, 4, 128)).astype(bf)
    # seed stationaries: [W_chunk; bias_chunk]
    wb = np.empty((2, 6, 128), np.float32)
    wb[0] = W.reshape(6, 128)
    bsum = b_i + b_r
    wb[1, 0:4] = bsum[:512].reshape(4, 128)
    wb[1, 4:6] = b_i[512:].reshape(2, 128)
    wb = wb.astype(bf)
    br3 = np.ascontiguousarray(b_r[512:].reshape(2, 128).T)  # [p, c]
    gb = np.empty((128, 4), np.float32)
    gb[:, 0:2] = ln_gamma.reshape(2, 128).T
    gb[:, 2:4] = ln_beta.reshape(2, 128).T
    wd = np.ascontiguousarray(Wd.reshape(2, 128, S).transpose(1, 0, 2))
    bdv = np.ascontiguousarray(bd.reshape(1, S))

    W3r = W[512:].reshape(2, 128)
    bi3r = b_i[512:].reshape(2, 128)
    in_maps = []
    for c in range(NCORES):
        xc = x[c * BC:(c + 1) * BC]  # (64, 1024)
        # x1[g, 0, t2, b] = x[b, 2*t2+g]; row 1 is the bias ones-row
        x1 = np.empty((2, 2, T // 2, BC), np.float32)
        x1[0, 0] = xc.T[0::2]
        x1[1, 0] = xc.T[1::2]
        x1[:, 1] = 1.0
        # xh3[p, t, c2, b] = W3[c2*128+p]*x[b, t] + b_i[512+c2*128+p]
        xh3 = (W3r.T[:, None, :, None] * xc.T[None, :, None, :]
               + bi3r.T[:, None, :, None]).astype(bf)
        in_maps.append({
            "x1": x1.astype(bf), "xh3": xh3, "ub": ub, "ubn": ubn, "wb": wb,
            "br3": br3, "gb": gb, "wd": wd, "bd": bdv,
        })

    trace = os.environ.get("GRU_TRACE", "") == "1"
    # The first execution of a freshly compiled NEFF occasionally hits a
    # transient NRT_EXEC_UNIT_UNRECOVERABLE on this stack; a retry succeeds.
    res = None
    last_err = None
    for attempt in range(3):
        try:
            res = run_bass_kernel_spmd(nc, in_maps, core_ids=list(range(NCORES)),
                                       trace=trace)
            break
        except Exception as e:  # noqa: BLE001
            last_err = e
    if res is None:
        raise last_err
    if trace:
        print(f"HW exec time: {res.exec_time_ns} ns")
        if res.instructions_and_trace:
            print(f"trace: {res.instructions_and_trace[1]}")
    out = np.concatenate([res.results[c]["out"] for c in range(NCORES)], axis=0)
    return out.astype(np.float32)



# revision 32
# speedup vs baseline: 1.1145x; 1.0014x over previous
# GRU summary kernel for Trainium2 (Bass/Tile), 8-core data-parallel over batch.
#
# Reference computation (see problem spec):
#   xp = x * W + b_i                      (rank-1 input projection, x scalar/step)
#   per t: rec = h @ U + b_r
#          z = sig(xp_z + rec_z); r = sig(xp_r + rec_r)
#          hh = tanh(xp_h + r * rec_h);  h = z*h + (1-z)*hh
#   out = LN(h) @ Wd + bd
#
# Layout: everything transposed ("f2"): state hT[p, c*64+b] = h[b, c*128+p],
# so matmul outputs (recT) land in [128-partition, batch-free] tiles and no
# per-step transposes are needed. U blocks are the stationary operand (bf16,
# FWL), hT is the moving operand. The rank-1 x-projection rides as K=2 seed
# matmuls with stationary [W_chunk; bias_chunk] and moving [x_t; 1].
#
# The per-step serial chain (the kernel is latency-bound, ~2.1-2.3us/step):
#   tanh -> u2=w*hh (DVE) -> 4 u2-mains (PE) -> sig_r (ACT) -> rr=r*rec_h
#   (DVE) -> ha=rr+xh (DVE) -> tanh ...
# Everything else (sig_z, w=1-z, v=z*h, h=v+u2, z/b-mains, seeds, DMA) is
# scheduled into the chain's engine-idle windows. Engine queue order is
# pinned with explicit dependency edges where the Tile scheduler's cost
# model would otherwise interleave off-chain ops into the chain (see the
# add_dependency block below).
import os
from contextlib import ExitStack

import numpy as np
import ml_dtypes

import concourse.bass as bass
import concourse.tile as tile
from concourse import bacc, mybir
from concourse.bass import ts
from concourse.bass_utils import run_bass_kernel_spmd

B, T, UH, S = 512, 1024, 256, 16
NCORES = 8
BC = B // NCORES  # 64 batch rows per core
QW = 512          # steps per window (fully unrolled inside For_i body)
NCH = 8           # xh DMA chunks per window (pipelines the 16.8MB load;
                  # each dma_start costs ~600ns of serial Sync-sequencer
                  # descriptor-gen at window entry, so fewer+bigger chunks
                  # win once the window is long enough to hide transfers)
CH = QW // NCH
LN_EPS = 1e-3

F32 = mybir.dt.float32
BF16 = mybir.dt.bfloat16
AF = mybir.ActivationFunctionType
OP = mybir.AluOpType

# number of windows; For_i loops over these. Overridable for smoke tests.
NW = T // QW


def _build(nc: bacc.Bacc, nw: int, br3_zero: bool):
    t_total = nw * QW
    # x seed operand split across the two legal matmul partition bases
    # (moving-operand base must be 0/32/64): even t at partitions 0:2, odd t
    # at 64:66, plane t//2. Halves the per-partition pool charge vs a
    # [2, T, BC] layout (which overflows SBUF at QW=512).
    x1_d = nc.dram_tensor("x1", [2, 2, T // 2, BC], BF16, kind="ExternalInput")
    xh_d = nc.dram_tensor("xh3", [128, T, 2, BC], BF16, kind="ExternalInput")
    ub_d = nc.dram_tensor("ub", [128, 12, 128], BF16, kind="ExternalInput")
    ubn_d = nc.dram_tensor("ubn", [128, 4, 128], BF16, kind="ExternalInput")
    wb_d = nc.dram_tensor("wb", [2, 6, 128], BF16, kind="ExternalInput")
    br3_d = nc.dram_tensor("br3", [128, 2], F32, kind="ExternalInput")
    gb_d = nc.dram_tensor("gb", [128, 4], F32, kind="ExternalInput")
    wd_d = nc.dram_tensor("wd", [128, 2, S], F32, kind="ExternalInput")
    bd_d = nc.dram_tensor("bd", [1, S], F32, kind="ExternalInput")
    out_d = nc.dram_tensor("out", [BC, S], F32, kind="ExternalOutput")

    with ExitStack() as ctx:
        tc = ctx.enter_context(tile.TileContext(nc))
        singles = ctx.enter_context(tc.tile_pool(name="singles", bufs=1))
        # bufs=1: For_i ends each window with an all-engine barrier, so
        # cross-window DMA/compute overlap is impossible anyway; one buffer
        # halves the SBUF footprint (QW=256 -> 64KB/partition for xh).
        xwin = ctx.enter_context(tc.tile_pool(name="xwin", bufs=1))
        psum = ctx.enter_context(tc.tile_pool(name="psum", bufs=2, space="PSUM"))
        psum1 = ctx.enter_context(tc.tile_pool(name="psum1", bufs=1, space="PSUM"))
        work = ctx.enter_context(tc.tile_pool(name="work", bufs=3))

        ub_s = singles.tile([128, 12, 128], BF16)
        nc.sync.dma_start(out=ub_s, in_=ub_d.ap())
        ubn_s = singles.tile([128, 4, 128], BF16)
        nc.sync.dma_start(out=ubn_s, in_=ubn_d.ap())
        # seed stationary replicated at partition bases 0 and 64 (matmul
        # requires stationary and moving operands to share a base partition)
        wb_s = singles.tile([128, 6, 128], BF16)
        nc.sync.dma_start(out=wb_s[0:2, :, :], in_=wb_d.ap())
        nc.sync.dma_start(out=wb_s[64:66, :, :], in_=wb_d.ap())
        br3_s = singles.tile([128, 2], F32)
        nc.sync.dma_start(out=br3_s, in_=br3_d.ap())
        gb_s = singles.tile([128, 4], F32)
        nc.sync.dma_start(out=gb_s, in_=gb_d.ap())
        wd_s = singles.tile([128, 2, S], F32)
        nc.sync.dma_start(out=wd_s, in_=wd_d.ap())
        bd_s = singles.tile([1, S], F32)
        nc.sync.dma_start(out=bd_s, in_=bd_d.ap())

        ones_r = singles.tile([1, 128], F32)
        nc.vector.memset(ones_r, 1.0)
        ones_c = singles.tile([128, 1], F32)
        nc.vector.memset(ones_c, 1.0)
        eps_s = singles.tile([1, 1], F32)
        nc.vector.memset(eps_s, LN_EPS)

        hb = singles.tile([128, 128], BF16)
        nc.vector.memset(hb, 0.0)
        v_prev = singles.tile([128, 128], BF16)
        nc.vector.memset(v_prev, 0.0)
        u2_prev = singles.tile([128, 128], BF16)
        nc.vector.memset(u2_prev, 0.0)

        # --- PE warm-up: ~30 back-to-back large matmuls so the HAM clock
        # gate opens (K=8/8, 2.4 GHz). The steady-state loop's PE idle gaps
        # are well under the ~3.4us MID window, so once warm it stays warm.
        # 14 x ~500ns (cold->mid clock) gives >3us of continuous PE work --
        # enough to open the clock gate; 30 was ~13us of serial prologue.
        warm_ps = psum1.tile([128, 512], F32, tag="warm")
        for _ in range(14):
            nc.tensor.matmul(warm_ps, ub_s[:, 0, :], ub_s[:, 0:4, :],
                             start=True, stop=True)

        def step(xs, xh, pz, pr, pb, sbase):
            # Seeds first: x-only deps, run during the previous gate chain.
            # NOTE: start=True clears has_written for the WHOLE bank -> exactly
            # one start=True per bank (its first write).
            for i, m in enumerate((2, 3)):
                nc.tensor.matmul(pr[:, i * 64:(i + 1) * 64],
                                 wb_s[sbase:sbase + 2, m, :], xs,
                                 start=(i == 0), stop=False, skip_group_check=True)
            for m in (0, 1):
                nc.tensor.matmul(pz[:, m * 64:(m + 1) * 64],
                                 wb_s[sbase:sbase + 2, m, :], xs,
                                 start=(m == 0), stop=False, skip_group_check=True)
            # r mains split via h_prev = v_prev + u2_prev (matmul linearity):
            # the v-part streams during the previous step's tanh; only the
            # u2-part (available right after tanh) sits on the serial chain.
            for kc in range(2):
                for i, m in enumerate((2, 3)):
                    nc.tensor.matmul(pr[:, i * 64:(i + 1) * 64],
                                     ub_s[:, 6 * kc + m, :],
                                     v_prev[:, kc * 64:(kc + 1) * 64],
                                     start=False, stop=False,
                                     skip_group_check=True)
            # u2_prev holds -u2 = (z-1)*hh (one fused stt op on the chain);
            # negated U blocks make the PSUM contribution (-U)^T(-u2) = +U^T u2.
            for kc in range(2):
                for i in range(2):
                    nc.tensor.matmul(pr[:, i * 64:(i + 1) * 64],
                                     ubn_s[:, 2 * kc + i, :],
                                     u2_prev[:, kc * 64:(kc + 1) * 64],
                                     start=False, stop=(kc == 1 and i == 1),
                                     skip_group_check=True)
            for kc in range(2):
                for i, m in enumerate((4, 5)):
                    nc.tensor.matmul(pb[:, i * 64:(i + 1) * 64],
                                     ub_s[:, 6 * kc + m, :],
                                     hb[:, kc * 64:(kc + 1) * 64],
                                     start=(kc == 0 and i == 0),
                                     stop=(kc == 1 and i == 1),
                                     skip_group_check=True)
            for kc in range(2):
                for m in (0, 1):
                    nc.tensor.matmul(pz[:, m * 64:(m + 1) * 64],
                                     ub_s[:, 6 * kc + m, :],
                                     hb[:, kc * 64:(kc + 1) * 64],
                                     start=False, stop=(kc == 1 and m == 1),
                                     skip_group_check=True)

            # --- gate chain: sig_r -> rrh -> ha -> tanh -> u2 -> (next r-mains)
            r_sb = work.tile([128, 128], BF16, tag="r")
            nc.scalar.activation(r_sb, pr, AF.Sigmoid)
            rr_sb = work.tile([128, 128], BF16, tag="rr")
            if br3_zero:
                nc.vector.tensor_mul(rr_sb, pb, r_sb)
            else:
                for c in range(2):
                    nc.vector.scalar_tensor_tensor(
                        rr_sb[:, c * 64:(c + 1) * 64], pb[:, c * 64:(c + 1) * 64],
                        br3_s[:, c:c + 1], r_sb[:, c * 64:(c + 1) * 64],
                        op0=OP.add, op1=OP.mult)
            ha_i = None
            ha_sb = work.tile([128, 128], BF16, tag="ha")
            ha_i = nc.vector.tensor_add(ha_sb, rr_sb, xh)
            # sig_z emitted before tanh: its release (pz stop) lands
            # mid-chain, well before tanh's input is ready.
            z_sb = work.tile([128, 128], BF16, tag="z")
            sigz_i = nc.scalar.activation(z_sb, pz, AF.Sigmoid)
            hh_sb = work.tile([128, 128], BF16, tag="hh")
            tanh_i = nc.scalar.activation(hh_sb, ha_sb, AF.Tanh)

            # off-chain: v = z*h_prev (must land early enough for the next
            # step's v-mains); chain: u2n = (z-1)*hh = -u2 in ONE fused stt
            # (no separate w=1-z op, so only v occupies the DVE during tanh
            # and u2n issues right at tanh_end+sem); then h = v - u2n.
            v_i = nc.vector.tensor_mul(v_prev, z_sb, hb)
            u2_i = nc.vector.scalar_tensor_tensor(u2_prev, z_sb, 1.0, hh_sb,
                                                  op0=OP.subtract, op1=OP.mult)
            nc.vector.tensor_sub(hb, v_prev, u2_prev)
            # Pin the DVE queue order ha -> v -> u2n and the ACT order
            # sig_z -> tanh. Without these edges the Tile scheduler's cost
            # model sometimes slots v between rr and ha (+~300ns of chain per
            # step, measured on v1), or parks sig_z after tanh, stalling v.
            _dep = v_i.ins.get_dependency_info(v_i.ins.sync_dependency_names()[0])
            v_i.ins.add_dependency(ha_i.ins.name, _dep)
            u2_i.ins.add_dependency(v_i.ins.name, _dep)
            tanh_i.ins.add_dependency(sigz_i.ins.name, _dep)

            # Dummy matmuls (stationary = fresh chain tiles, so they schedule
            # into THIS step's chain window) keep the PE busy so the HAM clock
            # gate stays open (2.4 GHz): without them the whole chip drops to
            # a lower clock state (every ACT/DVE op measured ~1.2x slower).
            nc.tensor.matmul(warm_ps, r_sb, ub_s[:, 0:4, :], start=True, stop=True)
            nc.tensor.matmul(warm_ps, r_sb, ub_s[:, 4:8, :], start=True, stop=True)
            nc.tensor.matmul(warm_ps, z_sb, ub_s[:, 0:4, :], start=True, stop=True)
            nc.tensor.matmul(warm_ps, z_sb, ub_s[:, 4:8, :], start=True, stop=True)

        with tc.For_i(0, nw, hint_engines=(mybir.EngineType.PE,
                                            mybir.EngineType.Activation,
                                            mybir.EngineType.DVE)) as w:
            xw = xwin.tile([128, QW // 2, BC], BF16, tag="xw")
            nc.sync.dma_start(out=xw[0:2, :, :],
                              in_=x1_d.ap()[0, :, ts(w, QW // 2), :])
            nc.sync.dma_start(out=xw[64:66, :, :],
                              in_=x1_d.ap()[1, :, ts(w, QW // 2), :])
            # xh window load split into NCH chunks so step q only waits for
            # chunk q//CH (the 8.4MB whole-window load cost ~30us serial).
            xh_tiles = []
            for k in range(NCH):
                t3 = xwin.tile([128, CH, 2, BC], BF16, tag=f"xh{k}")
                nc.sync.dma_start(out=t3, in_=xh_d.ap()[:, ts(w * NCH + k, CH), :, :])
                xh_tiles.append(t3)
            for q in range(QW):
                pz = psum.tile([128, 128], F32, tag="pz")
                pr = psum.tile([128, 128], F32, tag="pr")
                pb = psum.tile([128, 128], F32, tag="pb")
                base = 64 * (q % 2)
                step(xw[base:base + 2, q // 2, :],
                     xh_tiles[q // CH][:, q % CH, :, :], pz, pr, pb, base)

        # ---- epilogue: LayerNorm over hidden dim (partition axis) + dense
        ones_cb = singles.tile([128, 1], BF16)
        nc.vector.memset(ones_cb, 1.0)
        sq = work.tile([128, 128], F32, tag="sq")
        nc.vector.tensor_mul(sq, hb, hb)
        ps1 = psum.tile([1, 128], F32, tag="pz")
        nc.tensor.matmul(ps1, ones_cb, hb, start=True, stop=True)
        ps2 = psum.tile([1, 128], F32, tag="pb")
        nc.tensor.matmul(ps2, ones_c, sq, start=True, stop=True)

        s1_sb = work.tile([1, 128], F32, tag="s1")
        nc.vector.tensor_copy(s1_sb, ps1)
        s2_sb = work.tile([1, 128], F32, tag="s2")
        nc.vector.tensor_copy(s2_sb, ps2)
        mean_r = work.tile([1, 64], F32, tag="mean")
        nc.vector.tensor_add(mean_r, s1_sb[0:1, 0:64], s1_sb[0:1, 64:128])
        nc.vector.tensor_scalar_mul(mean_r, mean_r, 1.0 / UH)
        msq_r = work.tile([1, 64], F32, tag="msq")
        nc.vector.tensor_add(msq_r, s2_sb[0:1, 0:64], s2_sb[0:1, 64:128])
        nc.vector.tensor_scalar_mul(msq_r, msq_r, 1.0 / UH)
        m2_r = work.tile([1, 64], F32, tag="m2")
        nc.vector.tensor_mul(m2_r, mean_r, mean_r)
        var_r = work.tile([1, 64], F32, tag="var")
        nc.vector.tensor_sub(var_r, msq_r, m2_r)
        std_r = work.tile([1, 64], F32, tag="std")
        nc.scalar.activation(std_r, var_r, AF.Sqrt, bias=eps_s)
        rstd_r = work.tile([1, 64], F32, tag="rstd")
        nc.vector.reciprocal(rstd_r, std_r)

        pk = work.tile([1, 128], F32, tag="pk")
        nc.vector.tensor_copy(pk[0:1, 0:64], mean_r)
        nc.vector.tensor_copy(pk[0:1, 64:128], rstd_r)
        pbc = psum.tile([128, 128], F32, tag="pr")
        nc.tensor.matmul(pbc, ones_r, pk, start=True, stop=True)

        hn = work.tile([128, 128], F32, tag="hn")
        for c in range(2):
            t1 = work.tile([128, 64], F32, tag="t1")
            nc.vector.tensor_sub(t1, hb[:, c * 64:(c + 1) * 64], pbc[:, 0:64])
            t2 = work.tile([128, 64], F32, tag="t2")
            nc.vector.tensor_mul(t2, t1, pbc[:, 64:128])
            nc.vector.tensor_scalar(hn[:, c * 64:(c + 1) * 64], t2,
                                    gb_s[:, c:c + 1], gb_s[:, 2 + c:3 + c],
                                    op0=OP.mult, op1=OP.add)

        pd = psum.tile([64, S], F32, tag="pz")
        nc.tensor.matmul(pd, hn[:, 0:64], wd_s[:, 0, :], start=True, stop=False)
        nc.tensor.matmul(pd, hn[:, 64:128], wd_s[:, 1, :], start=False, stop=False)
        nc.tensor.matmul(pd, ones_r[0:1, 0:64], bd_s, start=False, stop=True)
        ob = work.tile([64, S], F32, tag="ob")
        nc.vector.tensor_copy(ob, pd)
        nc.sync.dma_start(out=out_d.ap(), in_=ob)


def kernel(**inputs) -> np.ndarray:
    x = np.asarray(inputs["time_series"], np.float32)[:, :, 0]  # (512, 1024)
    W = np.asarray(inputs["W"], np.float32)[0]                  # (768,)
    U = np.asarray(inputs["U"], np.float32)                     # (256, 768)
    b_i = np.asarray(inputs["b_i"], np.float32)
    b_r = np.asarray(inputs["b_r"], np.float32)
    ln_gamma = np.asarray(inputs["ln_gamma"], np.float32)
    ln_beta = np.asarray(inputs["ln_beta"], np.float32)
    Wd = np.asarray(inputs["Wd"], np.float32)
    bd = np.asarray(inputs["bd"], np.float32)

    nw = int(os.environ.get("GRU_NW", NW))
    br3_zero = not np.any(b_r[512:768])

    nc = bacc.Bacc("TRN2", target_bir_lowering=False, debug=False,
                   enable_asserts=True, num_devices=NCORES)
    _build(nc, nw, br3_zero)
    nc.compile()

    bf = ml_dtypes.bfloat16
    # U blocks: ub[p, kc*6+m, j] = U[kc*128+p, m*128+j]
    ub4 = U.reshape(2, 128, 6, 128).transpose(1, 0, 2, 3)
    ub = np.ascontiguousarray(ub4.reshape(128, 12, 128)).astype(bf)
    # negated r-gate blocks (m=2,3) for the u2n matmuls: (-U)^T(-u2) = U^T u2
    ubn = np.ascontiguousarray((-ub4[:, :, 2:4, :]).reshape(128, 4, 128)).astype(bf)
    # seed stationaries: [W_chunk; bias_chunk]
    wb = np.empty((2, 6, 128), np.float32)
    wb[0] = W.reshape(6, 128)
    bsum = b_i + b_r
    wb[1, 0:4] = bsum[:512].reshape(4, 128)
    wb[1, 4:6] = b_i[512:].reshape(2, 128)
    wb = wb.astype(bf)
    br3 = np.ascontiguousarray(b_r[512:].reshape(2, 128).T)  # [p, c]
    gb = np.empty((128, 4), np.float32)
    gb[:, 0:2] = ln_gamma.reshape(2, 128).T
    gb[:, 2:4] = ln_beta.reshape(2, 128).T
    wd = np.ascontiguousarray(Wd.reshape(2, 128, S).transpose(1, 0, 2))
    bdv = np.ascontiguousarray(bd.reshape(1, S))

    W3r = W[512:].reshape(2, 128)
    bi3r = b_i[512:].reshape(2, 128)
    in_maps = []
    for c in range(NCORES):
        xc = x[c * BC:(c + 1) * BC]  # (64, 1024)
        # x1[g, 0, t2, b] = x[b, 2*t2+g]; row 1 is the bias ones-row
        x1 = np.empty((2, 2, T // 2, BC), np.float32)
        x1[0, 0] = xc.T[0::2]
        x1[1, 0] = xc.T[1::2]
        x1[:, 1] = 1.0
        # xh3[p, t, c2, b] = W3[c2*128+p]*x[b, t] + b_i[512+c2*128+p]
        xh3 = (W3r.T[:, None, :, None] * xc.T[None, :, None, :]
               + bi3r.T[:, None, :, None]).astype(bf)
        in_maps.append({
            "x1": x1.astype(bf), "xh3": xh3, "ub": ub, "ubn": ubn, "wb": wb,
            "br3": br3, "gb": gb, "wd": wd, "bd": bdv,
        })

    trace = os.environ.get("GRU_TRACE", "") == "1"
    # The first execution of a freshly compiled NEFF occasionally hits a
    # transient NRT_EXEC_UNIT_UNRECOVERABLE on this stack; a retry succeeds.
    res = None
    last_err = None
    for attempt in range(3):
        try:
            res = run_bass_kernel_spmd(nc, in_maps, core_ids=list(range(NCORES)),
                                       trace=trace)
            break
        except Exception as e:  # noqa: BLE001
            last_err = e
    if res is None:
        raise last_err
    if trace:
        print(f"HW exec time: {res.exec_time_ns} ns")
        if res.instructions_and_trace:
            print(f"trace: {res.instructions_and_trace[1]}")
    out = np.concatenate([res.results[c]["out"] for c in range(NCORES)], axis=0)
    return out.astype(np.float32)

